# revision 7
# baseline (speedup 1.0000x reference)
"""Trainium2 Bass kernel for nn_Attention additive-attention problem.

Computation (reference, fp32):
    q = query @ Wq.T + bq                      # [B, H]
    r = ref @ Wr.T + br                        # [B, S, H]
    logits = einsum('bsh,h->bs', tanh(q[:,None,:] + r), V)
    w = softmax(logits, axis=1)                # over S
    out = einsum('bsh,bs->bh', r, w)[:, :, None]

Key identity used: since sum_s w = 1,
    out = (sum_s w_s * ref[s,:]) @ Wr.T + br
so r is only needed inside the tanh; the output reduction runs on ref
directly.

End-to-end latency budget (what kernel() wall time is made of): the
device executes in ~0.2-2 ms, while shipping `ref` to device HBM over
the axon tunnel runs at ~130 MB/s with a ~0.15 s fixed cost PER
TRANSFER (transfers serialize; measured 8x8MB = 1.6 s but 1x64MB =
0.62 s).  Host-side distribution strategy follows from that
measurement, not from FLOPs:

  - `ref` is quantized to fp8 e4m3 on the host (jax-cpu, ~0.2 s) and
    shipped as ONE 64 MB buffer to core 0; all 32 batches compute
    there (device ~2 ms -- noise at wall scale).  Spreading the bytes
    over 8 cores would mean 8 transfers and ~1 s MORE wall time.
  - The PJRT executable is built and warmed once at import; per call
    we only pay transfer + dispatch (the stock run_bass_kernel_spmd
    axon path re-traces and re-jits on every call).

On-chip dataflow per batch (4096 x 512):
  - fp8 natural-layout load -> DVE copy casts fp8->bf16 -> bf16 written
    back to a DRAM scratch tile -> one xbar DMA transpose per chunk
    loads refT[h%128, h//128, s] for the PE main matmuls.  The DVE
    weighted ref sum reads the bf16 natural tile straight from SBUF.
  - Main PE matmul r^T = WrT.T @ refT; ACT applies tanh with the
    per-partition bias qq = q + bq + br fused in.
  - logits^T come from PE matmuls with the tanh tile as stationary and V
    as a 1-column moving operand, so the softmax weights land with s on
    partitions; ACT exp emits them unnormalized.
  - The weighted ref sum runs as 4 tensor_scalar muls + 3 adds per s-tile
    on the otherwise idle DVE, then one 128->1 ones-matmul per s-tile
    accumulates into a batch-long PSUM bank.  Those matmuls are emitted
    two s-tiles late so the PE FIFO never waits on the DVE.
  - Batch epilogues (softmax denominator, normalization, projection
    through WrT + br) are deferred into the next batch's instruction
    stream for the same reason.

Numerics: fp8 e4m3 (TRN variant, max 240) storage of ref gives
rel_err ~9e-3 against the fp32 reference (simulated on the actual
input distribution); the bf16 on-chip arithmetic adds ~2e-3.  Gate is
2e-2.
"""

import numpy as np
import ml_dtypes
from contextlib import ExitStack

import concourse.bass as bass
import concourse.bacc as bacc
import concourse.tile as tile
from concourse import mybir
import concourse.bass_isa as bass_isa
from concourse._compat import with_exitstack

F32 = mybir.dt.float32
BF16 = mybir.dt.bfloat16
FP8 = mybir.dt.float8e4
AF = mybir.ActivationFunctionType
ALU = mybir.AluOpType
PSUM = bass.MemorySpace.PSUM

NP_FP8 = ml_dtypes.float8_e4m3          # mybir.dt.np(float8e4)
NP_BF16 = ml_dtypes.bfloat16

B, S, H = 32, 4096, 512
BPC = B                    # all batches on the one compute core
ST = 512                   # s-tile width
NST = S // ST              # s-tiles per batch = 8
NSC = S // 128             # 128-wide s-chunks per batch = 32
HC = H // 128              # h (and o) chunks = 4


@with_exitstack
def _body(ctx: ExitStack, tc: tile.TileContext,
          refq, qt, wq_c, wr_c, v_c, bq_c, br_c, br_f, out):
    nc = tc.nc

    consts = ctx.enter_context(tc.tile_pool(name="consts", bufs=1))
    nat8_pool = ctx.enter_context(tc.tile_pool(name="nat8", bufs=1))
    nat_pool = ctx.enter_context(tc.tile_pool(name="nat", bufs=2))
    refT_pool = ctx.enter_context(tc.tile_pool(name="refT", bufs=2))
    tanh_pool = ctx.enter_context(tc.tile_pool(name="tanh", bufs=3))
    wb_pool = ctx.enter_context(tc.tile_pool(name="wb", bufs=2))
    small = ctx.enter_context(tc.tile_pool(name="small", bufs=2))
    wn_pool = ctx.enter_context(tc.tile_pool(name="wn", bufs=5))
    q_pool = ctx.enter_context(tc.tile_pool(name="q", bufs=9))
    scratch = ctx.enter_context(tc.tile_pool(name="scratch", bufs=3, space="DRAM"))
    rps = ctx.enter_context(tc.tile_pool(name="rps", bufs=4, space=PSUM))
    lps = ctx.enter_context(tc.tile_pool(name="lps", bufs=1, space=PSUM))
    acc = ctx.enter_context(tc.tile_pool(name="acc", bufs=3, space=PSUM))

    # ---------------- prologue: params on chip (host pre-chunked layouts,
    # all contiguous DMAs, ~1.2 MiB total -- cheap, and the main matmuls
    # need the weights before anything else)
    wrt_bf = consts.tile([128, HC, H], BF16)   # WrT[h,o] as [h%128, hc, o]
    nc.sync.dma_start(wrt_bf[:], wr_c[:])
    wq_bf = consts.tile([128, HC, H], BF16)
    nc.sync.dma_start(wq_bf[:], wq_c[:])
    qt_bf = consts.tile([128, HC, BPC], BF16)
    nc.sync.dma_start(qt_bf[:], qt[:])
    v_bf = consts.tile([128, HC], BF16)        # V as [o%128, oc]
    nc.sync.dma_start(v_bf[:], v_c[:])
    bq_sb = consts.tile([128, HC], F32)
    nc.sync.dma_start(bq_sb[:], bq_c[:])
    br_sb = consts.tile([128, HC], F32)
    nc.sync.dma_start(br_sb[:], br_c[:])
    br_row = consts.tile([1, H], F32)
    nc.sync.dma_start(br_row[:], br_f[None, :])

    qq_sb = consts.tile([128, HC, BPC], F32)   # (q + bq + br)^T as [o%128, oc, b]
    ident = consts.tile([1, 1], F32)
    nc.gpsimd.memset(ident[:], 1.0)
    ones_bf = consts.tile([128, 1], BF16)
    nc.gpsimd.memset(ones_bf[:], 1.0)
    bqbr = consts.tile([128, HC], F32)
    nc.vector.tensor_add(bqbr[:], bq_sb[:], br_sb[:])

    # qq^T[o, b] = sum_h WqT[h, o] * queryT[h, b]  (+ bq + br)
    for oc in range(HC):
        qps = lps.tile([128, BPC], F32, tag="lt")
        for hc in range(HC):
            nc.tensor.matmul(
                qps[:],
                wq_bf[:, hc, oc * 128:(oc + 1) * 128],
                qt_bf[:, hc, :],
                start=(hc == 0),
                stop=(hc == HC - 1),
            )
        nc.vector.tensor_scalar_add(qq_sb[:, oc, :], qps[:], bqbr[:, oc:oc + 1])

    # ---------------- main loop ----------------
    def emit_epilogue(bb, wt_b, t_ps):
        """Softmax denom + projection for batch bb.  Emitted in the middle
        of batch bb+1's instruction stream so the strict PE FIFO never
        stalls waiting for the weight accumulation."""
        dsum = small.tile([128, 1], F32, tag="dsum")
        nc.vector.reduce_sum(dsum[:], wt_b[:], axis=mybir.AxisListType.X)
        dall = small.tile([128, 1], F32, tag="dall")
        nc.gpsimd.partition_all_reduce(dall[:], dsum[:], 128, bass_isa.ReduceOp.add)
        rec = small.tile([128, 1], F32, tag="rec")
        nc.vector.reciprocal(rec[:], dall[:])

        # normalize while evicting (scale = 1/D, fp32)
        t_sb = small.tile([1, H], F32, tag="t_sb")
        nc.scalar.activation(t_sb[:], t_ps[:], AF.Copy, scale=rec[0:1, 0:1])

        # transpose t to [h, 1] columns for the final projection
        tT_bf = small.tile([128, HC], BF16, tag="tT")
        for c in range(HC):
            ttp = acc.tile([128, 1], F32, tag="acc")
            nc.tensor.transpose(ttp[:], t_sb[0:1, c * 128:(c + 1) * 128], ident[0:1, 0:1])
            nc.vector.tensor_copy(tT_bf[:, c:c + 1], ttp[:])

        # out[1, o] = sum_h WrT[h, o] * t[h]  + br
        o_ps = acc.tile([1, H], F32, tag="acc")
        for c in range(HC):
            nc.tensor.matmul(
                o_ps[:],
                tT_bf[:, c:c + 1],
                wrt_bf[:, c, :],
                start=(c == 0),
                stop=(c == HC - 1),
            )
        out_sb = small.tile([1, H], F32, tag="out_sb")
        nc.vector.tensor_tensor(out_sb[:], o_ps[:], br_row[:], op=ALU.add)
        nc.sync.dma_start(out[bb:bb + 1, :], out_sb[:])

    def emit_stage(bb):
        """fp8 HBM -> bf16 SBUF natural tile + bf16 DRAM scratch copy.

        The DVE does the fp8->bf16 cast (its data converters handle fp8);
        the scratch DRAM tile then feeds the xbar transpose DMAs, which
        only accept 16-bit dtypes."""
        nat8 = nat8_pool.tile([128, NSC, H], FP8, tag="nat8", name=f"nat8_{bb}")
        nc.sync.dma_start(
            nat8[:], refq[bb].rearrange("(i p) h -> p i h", p=128)
        )
        nat = nat_pool.tile([128, NSC, H], BF16, tag="nat", name=f"nat_{bb}")
        nc.vector.tensor_copy(nat[:], nat8[:])
        sc = scratch.tile([S, H], BF16, tag="sc", name=f"sc_{bb}")
        nc.sync.dma_start(
            sc[:].rearrange("(i p) h -> p i h", p=128), nat[:]
        )
        return nat, sc

    def emit_transposes(bb, sc):
        refT = refT_pool.tile([128, HC, S], BF16, tag="refT", name=f"refT_{bb}")
        bounds = [0, 512, 1024, 2048, 3072, 4096] if bb == 0 else [0, 2048, 4096]
        for lo, hi in zip(bounds[:-1], bounds[1:]):
            nc.sync.dma_start(
                refT[:, :, lo:hi],
                sc[lo:hi, :],
                transpose=True,
            )
        return refT

    pending = None
    nat_next, sc_next = emit_stage(0)
    refT_next = emit_transposes(0, sc_next)
    for bb in range(BPC):
        nat, refT = nat_next, refT_next
        # next batch's staging + transposes go on the SP FIFO BEFORE this
        # batch's compute consumes its tiles, so the DMA queue stays ahead
        if bb + 1 < BPC:
            nat_next, sc_next = emit_stage(bb + 1)
            refT_next = emit_transposes(bb + 1, sc_next)

        wt_b = wb_pool.tile([128, NST * 4], F32)   # exp(logits)^T, [s%128, s//128]
        t_ps = acc.tile([1, H], F32, tag="acc")    # weighted ref sum (whole batch)
        mm_queue = []                              # deferred ones-matmuls

        def flush_mm(upto, t_ps=t_ps, mm_queue=mm_queue):
            while len(mm_queue) > upto:
                st_i, q4t = mm_queue.pop(0)
                nc.tensor.matmul(
                    t_ps[:],
                    ones_bf[:],
                    q4t[:],
                    start=(st_i == 0),
                    stop=(st_i == NST - 1),
                )

        def emit_logits_weights(st, tanh_prev, nat=nat, wt_b=wt_b,
                                mm_queue=mm_queue, flush_mm=None, bb=bb):
            # logits^T[s, 1] per 128-s sub-chunk: stationary = tanh tile.
            # Runs one tile behind the mains so its 16 weight loads prefetch
            # through the PE reorder window during the main streams.
            lt = lps.tile([128, 4], F32, tag="lt", name=f"lt_{bb}_{st}")
            for j in range(4):
                for oc in range(HC):
                    nc.tensor.matmul(
                        lt[:, j:j + 1],
                        tanh_prev[:, oc, j * 128:(j + 1) * 128],
                        v_bf[:, oc:oc + 1],
                        start=(oc == 0),
                        stop=(oc == HC - 1),
                    )
            nc.scalar.activation(wt_b[:, st * 4:(st + 1) * 4], lt[:], AF.Exp)

            # weighted ref rows on DVE: 4 per-chunk muls + add tree
            wn0 = wn_pool.tile([128, ST], BF16, tag="wn")
            nc.vector.tensor_scalar_mul(wn0[:], nat[:, st * 4 + 0, :], wt_b[:, st * 4 + 0:st * 4 + 1])
            wn1 = wn_pool.tile([128, ST], BF16, tag="wn")
            nc.vector.tensor_scalar_mul(wn1[:], nat[:, st * 4 + 1, :], wt_b[:, st * 4 + 1:st * 4 + 2])
            wn2 = wn_pool.tile([128, ST], BF16, tag="wn")
            nc.vector.tensor_scalar_mul(wn2[:], nat[:, st * 4 + 2, :], wt_b[:, st * 4 + 2:st * 4 + 3])
            wn3 = wn_pool.tile([128, ST], BF16, tag="wn")
            nc.vector.tensor_scalar_mul(wn3[:], nat[:, st * 4 + 3, :], wt_b[:, st * 4 + 3:st * 4 + 4])
            s01 = wn_pool.tile([128, ST], BF16, tag="sp")
            nc.vector.tensor_add(s01[:], wn0[:], wn1[:])
            s23 = wn_pool.tile([128, ST], BF16, tag="sp")
            nc.vector.tensor_add(s23[:], wn2[:], wn3[:])
            q4 = q_pool.tile([128, ST], BF16, tag="q")
            nc.vector.tensor_add(q4[:], s01[:], s23[:])
            mm_queue.append((st, q4))
            # keep the cross-partition reduction behind the DVE (batch 0's
            # nat load lands late, so defer its reductions to the end)
            flush_mm(NST)

        prev_tanh = None
        for st in range(NST):
            if st == 2 and pending is not None:
                emit_epilogue(*pending)
                pending = None
            # main matmul r^T[o, s] (+ tanh w/ bias on ACT)
            tanh_t = tanh_pool.tile([128, HC, ST], BF16)
            for oc in range(HC):
                ps = rps.tile([128, ST], F32)
                for hc in range(HC):
                    nc.tensor.matmul(
                        ps[:],
                        wrt_bf[:, hc, oc * 128:(oc + 1) * 128],
                        refT[:, hc, st * ST:(st + 1) * ST],
                        start=(hc == 0),
                        stop=(hc == HC - 1),
                    )
                nc.scalar.activation(
                    tanh_t[:, oc, :], ps[:], AF.Tanh, bias=qq_sb[:, oc, bb:bb + 1]
                )
            if prev_tanh is not None:
                emit_logits_weights(st - 1, prev_tanh, flush_mm=flush_mm)
            prev_tanh = tanh_t

        emit_logits_weights(NST - 1, prev_tanh, flush_mm=flush_mm)
        flush_mm(0)
        pending = (bb, wt_b, t_ps)

    emit_epilogue(*pending)


_NC_CACHE = None


def build_nc():
    global _NC_CACHE
    if _NC_CACHE is not None:
        return _NC_CACHE
    nc = bacc.Bacc("TRN2", target_bir_lowering=False, debug=False)
    refq = nc.dram_tensor("refq", [BPC, S, H], FP8, kind="ExternalInput").ap()
    qt = nc.dram_tensor("qt", [128, HC, BPC], BF16, kind="ExternalInput").ap()
    wq_c = nc.dram_tensor("wq_c", [128, HC, H], BF16, kind="ExternalInput").ap()
    wr_c = nc.dram_tensor("wr_c", [128, HC, H], BF16, kind="ExternalInput").ap()
    v_c = nc.dram_tensor("v_c", [128, HC], BF16, kind="ExternalInput").ap()
    bq_c = nc.dram_tensor("bq_c", [128, HC], F32, kind="ExternalInput").ap()
    br_c = nc.dram_tensor("br_c", [128, HC], F32, kind="ExternalInput").ap()
    br_f = nc.dram_tensor("br_f", [H], F32, kind="ExternalInput").ap()
    out = nc.dram_tensor("out", [BPC, H], F32, kind="ExternalOutput").ap()
    with tile.TileContext(nc) as tc:
        _body(tc, refq, qt, wq_c, wr_c, v_c, bq_c, br_c, br_f, out)
    nc.compile()
    _NC_CACHE = nc
    return nc


def _chunk_po(x):
    """[H(=hc*128+p), N] -> [128, HC, N] (pure layout)."""
    x = np.asarray(x)
    return np.ascontiguousarray(x.reshape(HC, 128, -1).transpose(1, 0, 2))


def make_small_inputs(query, Wq, bq, Wr, br, V):
    """Host-side layout marshalling for everything except ref (all tiny)."""
    query = np.asarray(query, np.float32)
    return {
        "qt": _chunk_po(query.T).astype(NP_BF16),        # [128, HC, B]
        "wq_c": _chunk_po(np.asarray(Wq, np.float32).T).astype(NP_BF16),
        "wr_c": _chunk_po(np.asarray(Wr, np.float32).T).astype(NP_BF16),
        "v_c": np.ascontiguousarray(
            np.asarray(V, np.float32).reshape(HC, 128).T).astype(NP_BF16),
        "bq_c": np.ascontiguousarray(np.asarray(bq, np.float32).reshape(HC, 128).T),
        "br_c": np.ascontiguousarray(np.asarray(br, np.float32).reshape(HC, 128).T),
        "br_f": np.ascontiguousarray(np.asarray(br, np.float32)),
    }


# ---------------------------------------------------------------------------
# PJRT runner.  Functionally the single-core axon path of
# bass_utils.run_bass_kernel_spmd -> bass2jax.run_bass_via_pjrt, but the
# traced/jitted executable is built ONCE and cached: the stock path creates a
# fresh closure per call, so jax re-traces and re-compiles on every kernel()
# invocation (~3 s/call of pure overhead at these shapes).
# ---------------------------------------------------------------------------

_RT = None


class _Runtime:
    def __init__(self):
        import jax
        import jax.numpy as jnp
        from concourse import bass2jax

        self.jax = jax
        self.jnp = jnp
        nc = build_nc()
        self.nc = nc
        bass2jax.install_neuronx_cc_hook()

        partition_name = (
            nc.partition_id_tensor.name if nc.partition_id_tensor else None
        )
        in_names, out_names, out_avals, zero_out_shapes = [], [], [], []
        for alloc in nc.m.functions[0].allocations:
            if not isinstance(alloc, mybir.MemoryLocationSet):
                continue
            name = alloc.memorylocations[0].name
            if alloc.kind == "ExternalInput":
                if name != partition_name and name != (
                    nc.dbg_addr.name if nc.dbg_addr is not None else None
                ):
                    in_names.append(name)
            elif alloc.kind == "ExternalOutput":
                shape = tuple(alloc.tensor_shape)
                dtype = mybir.dt.np(alloc.dtype)
                out_names.append(name)
                out_avals.append(jax.core.ShapedArray(shape, dtype))
                zero_out_shapes.append((shape, dtype))
        self.in_names = list(in_names)
        self.out_names = list(out_names)
        self.zero_out_shapes = zero_out_shapes
        n_params = len(in_names)
        all_names = in_names + out_names
        if partition_name is not None:
            all_names.append(partition_name)
        dbg_zero = None
        if nc.dbg_addr is not None:
            assert not nc.dbg_callbacks
            dbg_zero = np.zeros((1, 2), np.uint32)
            all_names.append(nc.dbg_addr.name)
        self.dbg_zero = dbg_zero
        out_avals_t = tuple(out_avals)
        all_names_t = tuple(all_names)
        out_names_t = tuple(out_names)

        def _raw_body(*args):
            operands = list(args)
            if partition_name is not None:
                operands.append(bass2jax.partition_id_tensor())
            if dbg_zero is not None:
                operands.append(jnp.asarray(dbg_zero))
            outs = bass2jax._bass_exec_p.bind(
                *operands,
                out_avals=out_avals_t,
                in_names=all_names_t,
                out_names=out_names_t,
                lowering_input_output_aliases=(),
                sim_require_finite=True,
                sim_require_nnan=True,
                nc=nc,
            )
            return tuple(outs)

        donate = tuple(range(n_params, n_params + len(out_names)))
        self.fn = jax.jit(_raw_body, donate_argnums=donate, keep_unused=True)
        self.dev = jax.devices()[0]

        # fp32 -> fp8 ref quantizer on the host CPU backend (multithreaded;
        # ~2.5x faster than np.ndarray.astype for 256 MB)
        self.cpu = jax.devices("cpu")[0]
        _q = jax.jit(lambda v: v.astype(NP_FP8))

        def quant(v):
            with jax.default_device(self.cpu):
                return _q(v)

        self.quant = quant

        # Warm everything once: XLA+neuronx compile, NEFF load, PJRT
        # dispatch, the host->device copy path, and the quantizer.  The
        # argument kinds must match real calls exactly (committed fp8 refq
        # on the device, uncommitted numpy for the small tensors) or the
        # first real call would re-trace under a different sharding key.
        # refq's dummy is built ON device (jnp.zeros) so the warmup ships
        # no 64 MB over the tunnel; a 1 MB device_put warms the transfer
        # path itself.
        shapes = {}
        for alloc in nc.m.functions[0].allocations:
            if isinstance(alloc, mybir.MemoryLocationSet):
                shapes[alloc.memorylocations[0].name] = (
                    tuple(alloc.tensor_shape), mybir.dt.np(alloc.dtype)
                )
        zero_in = {}
        for name in self.in_names:
            shape, dt = shapes[name]
            if name == "refq":
                zero_in[name] = jax.device_put(jnp.zeros(shape, dt), self.dev)
            else:
                zero_in[name] = np.zeros(shape, dt)
        self.run([zero_in[n] for n in self.in_names])
        jax.device_put(
            np.zeros(1 << 20, np.uint8), self.dev
        ).block_until_ready()
        np.asarray(self.quant(np.zeros((B, S, H), np.float32)))

    def run(self, inputs):
        zeros = [np.zeros(shape, dt) for shape, dt in self.zero_out_shapes]
        outs = self.fn(*inputs, *zeros)
        return {
            name: np.asarray(outs[i]) for i, name in enumerate(self.out_names)
        }


def _get_rt():
    global _RT
    if _RT is None:
        _RT = _Runtime()
    return _RT


def kernel(**inputs):
    rt = _get_rt()
    # Quantize ref on the CPU backend (async dispatch) while numpy marshals
    # the small tensors.
    refq_cpu = rt.quant(np.asarray(inputs["ref"], np.float32))
    small = make_small_inputs(
        inputs["query"], inputs["Wq"], inputs["bq"],
        inputs["Wr"], inputs["br"], inputs["V"],
    )
    # One big transfer for ref; the small tensors ride along in the jit call.
    feed = {"refq": rt.jax.device_put(np.asarray(refq_cpu), rt.dev)}
    feed.update(small)
    res = rt.run([feed[n] for n in rt.in_names])
    return np.asarray(res["out"], np.float32)[:, :, None]


# -- helpers kept for test.py compatibility ---------------------------------

def make_in_maps(query, ref, Wq, bq, Wr, br, V):
    m = make_small_inputs(query, Wq, bq, Wr, br, V)
    m["refq"] = np.asarray(ref, np.float32).astype(NP_FP8)
    return [m]


def run(query, ref, Wq, bq, Wr, br, V, trace=False):
    """Trace-capable path through bass_utils (used by test.py for NTFF)."""
    from concourse import bass_utils
    nc = build_nc()
    in_maps = make_in_maps(query, ref, Wq, bq, Wr, br, V)
    res = bass_utils.run_bass_kernel_spmd(
        nc, in_maps, core_ids=[0], trace=trace
    )
    full = np.asarray(res.results[0]["out"], np.float32)
    return full[:, :, None], res
